# revision 24
# baseline (speedup 1.0000x reference)
"""FAVOR+ linear attention (Performer-style) Trainium2 Bass kernel.

Full inputs -> full output. Sharding: 8 cores = (batch b in 0..3) x (query
half in 0..1). Each core computes the KV summary over all 16384 key pixels
of its batch (duplicated within the pair; avoids collectives) and the
numerator/denominator for its 8192 query pixels.

Engine balance: PE does all matmuls/transposes; Act does the feature exps
(+ half the v copies); Pool (gpsimd) does the PSUM->SBUF t copies; DVE does
t^2, half the v copies, and the output normalization. The q/k projection
biases are folded host-side into the feature matrices (bmat') plus a
per-feature constant that scales the KV-summary rows (csc into mask136),
so every PSUM->SBUF copy is a pure dtype-converting copy.

Hardcoded problem shape: B=4, C=128, H=W=128, hid=128, heads=8, hd=dv=16.
"""

import numpy as np
from contextlib import ExitStack

import concourse.bass as bass
import concourse.tile as tile
from concourse import bacc, mybir
from concourse.bass_utils import run_bass_kernel_spmd

F32 = mybir.dt.float32
F32R = mybir.dt.float32r
BF16 = mybir.dt.bfloat16
AF = mybir.ActivationFunctionType

N_HEADS = 8
HD = 16          # head dim for q/k and v
C = 128          # channels == hid
S = 128 * 128    # pixels per image
SQ = S // 2      # query pixels per core
SK = S           # key pixels per core (duplicated across the pair)
SC = 2048        # super-chunk (pixels)

_PROGRAM = None


def _build_program(debug=False, loop_n=None, seqpar=False):
    nc = bacc.Bacc()
    sk = SQ if seqpar else SK
    xs = nc.declare_dram_parameter("xs", [C, SQ], F32, isOutput=False)
    ys = nc.declare_dram_parameter("ys", [C, sk], F32, isOutput=False)
    cpack = nc.declare_dram_parameter("cpack", [128, 136], F32, isOutput=False)
    cpackb = nc.declare_dram_parameter("cpackb", [128, 392], BF16, isOutput=False)
    cpackr = nc.declare_dram_parameter("cpackr", [128, 384], F32R, isOutput=False)
    outp = nc.declare_dram_parameter("outp", [SQ, 128], F32, isOutput=True)
    outp_r = outp.rearrange("(n k p) c -> n p k c", k=SC // 128, p=128)

    with tile.TileContext(nc) as tc, ExitStack() as ctx:
        singles = ctx.enter_context(tc.tile_pool(name="singles", bufs=1))
        inpool = ctx.enter_context(tc.tile_pool(name="inpool", bufs=4))
        tpool = ctx.enter_context(tc.tile_pool(name="tpool", bufs=3))
        fpool = ctx.enter_context(tc.tile_pool(name="fpool", bufs=2))
        qpool = ctx.enter_context(tc.tile_pool(name="qpool", bufs=4))
        vpool = ctx.enter_context(tc.tile_pool(name="vpool", bufs=2))
        npool = ctx.enter_context(tc.tile_pool(name="npool", bufs=2))
        opool = ctx.enter_context(tc.tile_pool(name="opool", bufs=3))
        pps = ctx.enter_context(tc.tile_pool(name="pps", bufs=2, space="PSUM"))
        ppkv = ctx.enter_context(tc.tile_pool(name="ppkv", bufs=1, space="PSUM"))
        ppn = ctx.enter_context(tc.tile_pool(name="ppn", bufs=1, space="PSUM"))

        consts = singles.tile([128, 136], F32)
        nc.sync.dma_start(out=consts, in_=cpack[:])
        mask136 = consts[:, 0:136]
        constsb = singles.tile([128, 392], BF16)
        nc.sync.dma_start(out=constsb, in_=cpackb[:])
        bmq_bf = constsb[:, 0:128]
        bmk_bf = constsb[:, 128:256]
        fmat_bf = constsb[:, 256:384]
        fm8_bf = constsb[:, 384:392]
        constsr = singles.tile([128, 384], F32R)
        nc.sync.dma_start(out=constsr, in_=cpackr[:])
        wqt_r = constsr[:, 0:128]
        wkt_r = constsr[:, 128:256]
        ident_r = constsr[:, 256:384]

        # Tiny ops so each engine observes the consts DMAs once up front;
        # later instructions then carry at most one new semaphore wait each.
        presync = pps.tile([128, 512], F32, tag="tps")
        nc.tensor.matmul(presync[:1, 0:1], lhsT=consts[:, 0:1],
                         rhs=consts[:, 0:1], start=True, stop=True)
        dve_sync = singles.tile([128, 1], F32)
        nc.vector.tensor_copy(dve_sync, consts[:, 0:1])
        act_warm = singles.tile([128, 1], F32)
        nc.scalar.activation(act_warm, consts[:, 0:1], AF.Exp)
        pool_sync = singles.tile([128, 1], F32)
        nc.gpsimd.tensor_copy(pool_sync, consts[:, 0:1])

        def body():
            # ---- K phase: KV[hm, (h v | nrm-sum)] accumulated in PSUM.
            # The per-pixel-per-head factor exp(-|t_h|^2/2) is folded into V
            # (only same-head KV entries are kept by the mask), with a
            # per-head nrm column replacing the plain ones column.
            kvps = ppkv.tile([128, 136], F32, tag="kv")
            nkc = sk // SC
            qs_tiles = {}

            def emit_k_sc(isc):
                y_t = inpool.tile([128, SC], F32R, tag="inbuf")
                if isc == 0:
                    h = SC // 2
                    nc.sync.dma_start(out=y_t[:, 0:h], in_=ys[:, 0:h].bitcast(F32R))
                    nc.sync.dma_start(out=y_t[:, h:SC], in_=ys[:, h:SC].bitcast(F32R))
                else:
                    nc.sync.dma_start(out=y_t,
                                      in_=ys[:, isc * SC:(isc + 1) * SC].bitcast(F32R))
                t_t = tpool.tile([128, SC], BF16, tag="t")
                t2_t = tpool.tile([128, SC], BF16, tag="t2")
                for j in range(SC // 512):
                    blk = slice(j * 512, (j + 1) * 512)
                    tps = pps.tile([128, 512], F32, tag="tps")
                    nc.tensor.matmul(tps, lhsT=wkt_r, rhs=y_t[:, blk],
                                     start=True, stop=True)
                    if j % 2 == 0:
                        nc.scalar.activation(t_t[:, blk], tps, AF.Identity)
                    else:
                        nc.vector.tensor_copy(t_t[:, blk], tps)
                    nc.gpsimd.tensor_mul(t2_t[:, blk], t_t[:, blk], t_t[:, blk])
                kf_t = fpool.tile([128, SC], BF16, tag="feat")
                nrm_ps = ppn.tile([128, SC // 128, 8], F32, tag="nrm")
                kfps_q = []
                for j in range(SC // 512):
                    blk = slice(j * 512, (j + 1) * 512)
                    kfps = pps.tile([128, 512], F32, tag="kfps")
                    for c in range(4):
                        cc = j * 4 + c
                        sl = slice(c * 128, (c + 1) * 128)
                        ch = slice(cc * 128, (cc + 1) * 128)
                        nc.tensor.matmul(kfps[:, sl], lhsT=t_t[:, ch], rhs=bmk_bf,
                                         start=True, stop=True)
                        nc.tensor.matmul(nrm_ps[:, cc, :], lhsT=t2_t[:, ch],
                                         rhs=fm8_bf, start=True, stop=True)
                    kfps_q.append((blk, kfps))
                    if len(kfps_q) == 2 and j < 3:
                        b0, p0 = kfps_q.pop(0)
                        nc.scalar.activation(kf_t[:, b0], p0, AF.Exp)
                nrmb_t = vpool.tile([128, SC // 128, 8], BF16, tag="nrmb")
                nc.scalar.activation(nrmb_t, nrm_ps, AF.Exp)
                for b0, p0 in kfps_q:
                    nc.scalar.activation(kf_t[:, b0], p0, AF.Exp)
                v_t = vpool.tile([128, SC // 128, 8, 17], BF16, tag="v")
                nc.gpsimd.tensor_copy(v_t[:, :, :, 16], nrmb_t)
                for j in range(SC // 512):
                    blk = slice(j * 512, (j + 1) * 512)
                    vps = pps.tile([128, 512], F32R, tag="vps")
                    for c in range(4):
                        cc = j * 4 + c
                        sl = slice(c * 128, (c + 1) * 128)
                        ch = slice(cc * 128, (cc + 1) * 128)
                        nc.tensor.transpose(vps[:, sl], y_t[:, ch], ident_r)
                    j4 = slice(j * 4, (j + 1) * 4)
                    nc.vector.tensor_mul(
                        v_t[:, j4, :, 0:16],
                        vps.rearrange("p (c h f) -> p c h f", c=4, h=8),
                        nrmb_t[:, j4, :, None].to_broadcast([128, 4, 8, 16]))

                for cc in range(SC // 128):
                    ch = slice(cc * 128, (cc + 1) * 128)
                    gfirst = (isc == 0) and (cc == 0)
                    last = (isc == nkc - 1) and (cc == SC // 128 - 1)
                    nc.tensor.matmul(kvps, lhsT=kf_t[:, ch],
                                     rhs=v_t[:, cc].rearrange("p h j -> p (h j)"),
                                     start=gfirst, stop=last,
                                     skip_group_check=True)

            def emit_kvb():
                # mask to block-diagonal [KV | nrmsum] per head
                kvsb = npool.tile([128, 136], F32, tag="kvsb")
                nc.vector.tensor_copy(kvsb, kvps)
                kvb3 = npool.tile([128, 8, 17], BF16, tag="kvb3")
                m3 = mask136.rearrange("p (h j) -> p h j", h=8)
                nc.gpsimd.tensor_mul(kvb3,
                                     kvsb.rearrange("p (h j) -> p h j", h=8), m3)
                return kvb3.rearrange("p h j -> p (h j)")

            def emit_q_feat(isc):
                x_t = inpool.tile([128, SC], F32R, tag="xin")
                nc.sync.dma_start(out=x_t, in_=xs[:, isc * SC:(isc + 1) * SC].bitcast(F32R))
                t_t = tpool.tile([128, SC], BF16, tag="t")
                t2_t = tpool.tile([128, SC], BF16, tag="t2")
                for j in range(SC // 512):
                    blk = slice(j * 512, (j + 1) * 512)
                    tps = pps.tile([128, 512], F32, tag="tps")
                    nc.tensor.matmul(tps, lhsT=wqt_r, rhs=x_t[:, blk],
                                     start=True, stop=True)
                    if j != 1:
                        nc.scalar.activation(t_t[:, blk], tps, AF.Identity)
                    else:
                        nc.vector.tensor_copy(t_t[:, blk], tps)
                    nc.gpsimd.tensor_mul(t2_t[:, blk], t_t[:, blk], t_t[:, blk])
                qs_t = qpool.tile([128, SC], BF16, tag="qs")
                for j in range(SC // 512):
                    blk = slice(j * 512, (j + 1) * 512)
                    qps = pps.tile([128, 512], F32, tag="kfps")
                    nc.tensor.matmul(qps, lhsT=bmq_bf, rhs=t_t[:, blk],
                                     start=True, stop=False)
                    nc.tensor.matmul(qps, lhsT=fmat_bf, rhs=t2_t[:, blk],
                                     start=False, stop=True)
                    nc.scalar.activation(qs_t[:, blk], qps, AF.Exp)
                qs_tiles[isc] = qs_t

            def emit_q_num(isc, kvb):
                qs_t = qs_tiles.pop(isc)
                out_t = opool.tile([128, SC // 128, 128], F32, tag="outb")
                cc = 0
                while cc < SC // 128:
                    g = min(3, SC // 128 - cc)
                    nps = pps.tile([128, 3, 136], F32, tag="vps")
                    for i in range(g):
                        ch = slice((cc + i) * 128, (cc + i + 1) * 128)
                        nc.tensor.matmul(nps[:, i, :], lhsT=qs_t[:, ch], rhs=kvb,
                                         start=True, stop=True)
                    nps4 = nps[:, 0:g, :].rearrange("p c (h j) -> p c h j", h=8)
                    rden = npool.tile([128, 3, 8], F32, tag="rden")
                    nc.vector.reciprocal(rden[:, 0:g], nps4[:, :, :, 16])
                    nc.vector.tensor_mul(
                        out_t[:, cc:cc + g].rearrange("p c (h j) -> p c h j", h=8),
                        nps4[:, :, :, 0:16],
                        rden[:, 0:g, :, None].to_broadcast([128, g, 8, 16]))
                    cc += g
                    if cc == 9:
                        nc.sync.dma_start(out=outp_r[isc][:, 0:8], in_=out_t[:, 0:8])
                nc.sync.dma_start(out=outp_r[isc][:, 8:], in_=out_t[:, 8:])

            for isc in range(nkc):
                emit_k_sc(isc)
            kvb = emit_kvb()
            for isc in range(SQ // SC):
                emit_q_feat(isc)
                emit_q_num(isc, kvb)

        if loop_n is None:
            body()
        else:
            with tc.For_i(0, loop_n, 1):
                body()

    nc.compile()
    return nc


def _get_program():
    global _PROGRAM
    if _PROGRAM is None:
        _PROGRAM = _build_program(seqpar=SEQPAR)
    return _PROGRAM


def _host_consts(rfs, Wq, bq, Wk, bk):
    import ml_dtypes
    scale = HD ** -0.25  # == 0.5 exactly
    rfs = rfs.astype(np.float64)
    bmat = np.zeros((128, 128))
    fmat = np.zeros((128, 128))
    for h in range(N_HEADS):
        bmat[16 * h:16 * h + 16, 16 * h:16 * h + 16] = rfs[h]
        fmat[16 * h:16 * h + 16, 16 * h:16 * h + 16] = -0.5

    def fold(bvec):
        bp = scale * bvec.astype(np.float64)
        bm = bmat.copy()
        cc = np.zeros(128)
        for h in range(N_HEADS):
            sl = slice(16 * h, 16 * h + 16)
            bm[sl, sl] -= bp[sl][:, None]
            cc[sl] = bp @ bmat[:, sl] - 0.5 * np.sum(bp[sl] ** 2)
        return bm, cc

    bmq, cq = fold(bq)
    bmk, ck = fold(bk)
    csc = np.exp(cq + ck)

    mask = np.zeros((128, 136), dtype=np.float64)
    for h in range(N_HEADS):
        mask[16 * h:16 * h + 16, 17 * h:17 * h + 17] = 1.0
    mask *= csc[:, None]
    cpack = np.ascontiguousarray(mask, dtype=np.float32)

    fm8 = np.zeros((128, 8))
    for h in range(N_HEADS):
        fm8[16 * h:16 * h + 16, h] = -0.5
    cpackb = np.concatenate([bmq, bmk, fmat, fm8], axis=1).astype(ml_dtypes.bfloat16)
    cpackr = np.ascontiguousarray(np.concatenate(
        [(scale * Wq).T, (scale * Wk).T, np.eye(128)], axis=1), dtype=np.float32)
    return cpack, cpackb, cpackr


SEQPAR = False


def make_in_maps(inputs):
    x = np.ascontiguousarray(np.asarray(inputs["x"], dtype=np.float32))
    y = np.ascontiguousarray(np.asarray(inputs["y"], dtype=np.float32))
    cpack, cpackb, cpackr = _host_consts(np.asarray(inputs["rfs"], dtype=np.float32),
                         np.asarray(inputs["Wq"], dtype=np.float32),
                         np.asarray(inputs["bq"], dtype=np.float32),
                         np.asarray(inputs["Wk"], dtype=np.float32),
                         np.asarray(inputs["bk"], dtype=np.float32))
    B = x.shape[0]
    xr = x.reshape(B, C, S)
    yr = y.reshape(B, C, S)
    in_maps = []
    for core in range(8):
        b, half = core // 2, core % 2
        s0 = half * SQ
        ys_i = yr[b][:, s0:s0 + SQ] if SEQPAR else yr[b]
        in_maps.append({
            "xs": np.ascontiguousarray(xr[b][:, s0:s0 + SQ]),
            "ys": np.ascontiguousarray(ys_i),
            "cpack": cpack,
            "cpackb": cpackb,
            "cpackr": cpackr,
        })
    return in_maps


def run(inputs, trace=False, **kwargs):
    in_maps = make_in_maps(inputs)
    nc = _get_program()
    res = run_bass_kernel_spmd(nc, in_maps, list(range(8)), trace=trace, **kwargs)
    B = np.asarray(inputs["x"]).shape[0]
    out = np.empty((B, S, 128), dtype=np.float32)
    for core in range(8):
        b, half = core // 2, core % 2
        s0 = half * SQ
        out[b, s0:s0 + SQ, :] = res.results[core]["outp"]
    return out.reshape(np.asarray(inputs["x"]).shape), res


def kernel(**inputs):
    out, _ = run(inputs, trace=False)
    return out


# revision 26
# speedup vs baseline: 1.1038x; 1.1038x over previous
"""FAVOR+ linear attention (Performer-style) Trainium2 Bass kernel.

Full inputs -> full output. Sharding: 8 cores = (batch b in 0..3) x (query
half in 0..1). Each core computes the KV summary over all 16384 key pixels
of its batch (duplicated within the pair; avoids collectives) and the
numerator/denominator for its 8192 query pixels.

Engine balance: PE does all matmuls/transposes; Act does the feature exps
(+ half the v copies); Pool (gpsimd) does the PSUM->SBUF t copies; DVE does
t^2, half the v copies, and the output normalization. The q/k projection
biases are folded host-side into the feature matrices (bmat') plus a
per-feature constant that scales the KV-summary rows (csc into mask136),
so every PSUM->SBUF copy is a pure dtype-converting copy.

Hardcoded problem shape: B=4, C=128, H=W=128, hid=128, heads=8, hd=dv=16.
"""

import numpy as np
from contextlib import ExitStack

import concourse.bass as bass
import concourse.tile as tile
from concourse import bacc, mybir
from concourse.bass_utils import run_bass_kernel_spmd

F32 = mybir.dt.float32
F32R = mybir.dt.float32r
BF16 = mybir.dt.bfloat16
AF = mybir.ActivationFunctionType

N_HEADS = 8
HD = 16          # head dim for q/k and v
C = 128          # channels == hid
S = 128 * 128    # pixels per image
SQ = S // 2      # query pixels per core
SK = S           # key pixels per core (duplicated across the pair)
SC = 2048        # super-chunk (pixels)

_PROGRAM = None


def _build_program(debug=False, loop_n=None, seqpar=False):
    nc = bacc.Bacc()
    sk = SQ if seqpar else SK
    xs = nc.declare_dram_parameter("xs", [C, SQ], F32, isOutput=False)
    ys = nc.declare_dram_parameter("ys", [C, sk], F32, isOutput=False)
    cpack = nc.declare_dram_parameter("cpack", [128, 136], F32, isOutput=False)
    cpackb = nc.declare_dram_parameter("cpackb", [128, 392], BF16, isOutput=False)
    cpackr = nc.declare_dram_parameter("cpackr", [128, 384], F32R, isOutput=False)
    outp = nc.declare_dram_parameter("outp", [SQ, 128], F32, isOutput=True)
    outp_r = outp.rearrange("(n k p) c -> n p k c", k=SC // 128, p=128)

    with tile.TileContext(nc) as tc, ExitStack() as ctx:
        singles = ctx.enter_context(tc.tile_pool(name="singles", bufs=1))
        inpool = ctx.enter_context(tc.tile_pool(name="inpool", bufs=4))
        tpool = ctx.enter_context(tc.tile_pool(name="tpool", bufs=3))
        fpool = ctx.enter_context(tc.tile_pool(name="fpool", bufs=2))
        qpool = ctx.enter_context(tc.tile_pool(name="qpool", bufs=4))
        vpool = ctx.enter_context(tc.tile_pool(name="vpool", bufs=2))
        npool = ctx.enter_context(tc.tile_pool(name="npool", bufs=2))
        opool = ctx.enter_context(tc.tile_pool(name="opool", bufs=3))
        pps = ctx.enter_context(tc.tile_pool(name="pps", bufs=2, space="PSUM"))
        ppkv = ctx.enter_context(tc.tile_pool(name="ppkv", bufs=1, space="PSUM"))

        consts = singles.tile([128, 136], F32)
        nc.sync.dma_start(out=consts, in_=cpack[:])
        mask136 = consts[:, 0:136]
        constsb = singles.tile([128, 392], BF16)
        nc.sync.dma_start(out=constsb, in_=cpackb[:])
        bmq_bf = constsb[:, 0:128]
        bmk_bf = constsb[:, 128:256]
        fmat_bf = constsb[:, 256:384]
        fm8_bf = constsb[:, 384:392]
        constsr = singles.tile([128, 384], F32R)
        nc.sync.dma_start(out=constsr, in_=cpackr[:])
        wqt_r = constsr[:, 0:128]
        wkt_r = constsr[:, 128:256]
        ident_r = constsr[:, 256:384]

        # Tiny ops so each engine observes the consts DMAs once up front;
        # later instructions then carry at most one new semaphore wait each.
        presync = pps.tile([128, 512], F32, tag="tps")
        nc.tensor.matmul(presync[:1, 0:1], lhsT=consts[:, 0:1],
                         rhs=consts[:, 0:1], start=True, stop=True)
        dve_sync = singles.tile([128, 1], F32)
        nc.vector.tensor_copy(dve_sync, consts[:, 0:1])
        act_warm = singles.tile([128, 1], F32)
        nc.scalar.activation(act_warm, consts[:, 0:1], AF.Exp)
        pool_sync = singles.tile([128, 1], F32)
        nc.gpsimd.tensor_copy(pool_sync, consts[:, 0:1])

        def body():
            # ---- K phase: KV[hm, (h v | nrm-sum)] accumulated in PSUM.
            # The per-pixel-per-head factor exp(-|t_h|^2/2) is folded into V
            # (only same-head KV entries are kept by the mask), with a
            # per-head nrm column replacing the plain ones column.
            kvps = ppkv.tile([128, 129], F32, tag="kv")
            nkc = sk // SC
            qs_tiles = {}

            def emit_k_sc(isc):
                y_t = inpool.tile([128, SC], F32R, tag="inbuf")
                if isc == 0:
                    h = SC // 2
                    nc.sync.dma_start(out=y_t[:, 0:h], in_=ys[:, 0:h].bitcast(F32R))
                    nc.sync.dma_start(out=y_t[:, h:SC], in_=ys[:, h:SC].bitcast(F32R))
                else:
                    nc.sync.dma_start(out=y_t,
                                      in_=ys[:, isc * SC:(isc + 1) * SC].bitcast(F32R))
                t_t = tpool.tile([128, SC], BF16, tag="t")
                t2_t = tpool.tile([128, SC], BF16, tag="t2")
                for j in range(SC // 512):
                    blk = slice(j * 512, (j + 1) * 512)
                    tps = pps.tile([128, 512], F32, tag="tps")
                    nc.tensor.matmul(tps, lhsT=wkt_r, rhs=y_t[:, blk],
                                     start=True, stop=True)
                    if j < 2:
                        nc.scalar.activation(t_t[:, blk], tps, AF.Identity)
                    else:
                        nc.vector.tensor_copy(t_t[:, blk], tps)
                    nc.vector.tensor_mul(t2_t[:, blk], t_t[:, blk], t_t[:, blk])
                kf_t = fpool.tile([128, SC], BF16, tag="feat")
                v_t = vpool.tile([128, SC // 128, 129], BF16, tag="v")
                nc.gpsimd.memset(v_t[:, :, 128:129], 1.0)
                for j in range(SC // 512):
                    blk = slice(j * 512, (j + 1) * 512)
                    kfps = pps.tile([128, 512], F32, tag="kfps")
                    vps = pps.tile([128, 512], F32R, tag="vps")
                    for c in range(4):
                        cc = j * 4 + c
                        sl = slice(c * 128, (c + 1) * 128)
                        ch = slice(cc * 128, (cc + 1) * 128)
                        nc.tensor.matmul(kfps[:, sl], lhsT=t_t[:, ch], rhs=bmk_bf,
                                         start=True, stop=False)
                        nc.tensor.matmul(kfps[:, sl], lhsT=t2_t[:, ch], rhs=fmat_bf,
                                         start=False, stop=True)
                        nc.tensor.transpose(vps[:, sl], y_t[:, ch], ident_r)
                    nc.scalar.activation(kf_t[:, blk], kfps, AF.Exp)
                    vdst = v_t[:, j * 4:(j + 1) * 4, 0:128]
                    vsrc = vps.rearrange("p (c f) -> p c f", c=4)
                    if j < 3:
                        nc.vector.tensor_copy(vdst, vsrc)
                    else:
                        nc.scalar.activation(vdst, vsrc.bitcast(F32), AF.Identity)

                for cc in range(SC // 128):
                    ch = slice(cc * 128, (cc + 1) * 128)
                    gfirst = (isc == 0) and (cc == 0)
                    last = (isc == nkc - 1) and (cc == SC // 128 - 1)
                    nc.tensor.matmul(kvps, lhsT=kf_t[:, ch],
                                     rhs=v_t[:, cc, :], start=gfirst, stop=last,
                                     skip_group_check=True)

            def emit_kvb():
                # mask to block-diagonal [KV | ksum] per head
                kvsb = npool.tile([128, 129], F32, tag="kvsb")
                nc.vector.tensor_copy(kvsb, kvps)
                kvb3 = npool.tile([128, 8, 17], BF16, tag="kvb3")
                m3 = mask136.rearrange("p (h j) -> p h j", h=8)
                nc.vector.tensor_mul(kvb3[:, :, 0:16],
                                     kvsb[:, 0:128].rearrange("p (h j) -> p h j", h=8),
                                     m3[:, :, 0:16])
                nc.vector.tensor_mul(kvb3[:, :, 16:17],
                                     kvsb[:, 128:129, None].to_broadcast([128, 8, 1]),
                                     m3[:, :, 16:17])
                return kvb3.rearrange("p h j -> p (h j)")

            def emit_q_feat(isc):
                x_t = inpool.tile([128, SC], F32R, tag="xin")
                nc.sync.dma_start(out=x_t, in_=xs[:, isc * SC:(isc + 1) * SC].bitcast(F32R))
                t_t = tpool.tile([128, SC], BF16, tag="t")
                t2_t = tpool.tile([128, SC], BF16, tag="t2")
                for j in range(SC // 512):
                    blk = slice(j * 512, (j + 1) * 512)
                    tps = pps.tile([128, 512], F32, tag="tps")
                    nc.tensor.matmul(tps, lhsT=wqt_r, rhs=x_t[:, blk],
                                     start=True, stop=True)
                    nc.scalar.activation(t_t[:, blk], tps, AF.Identity)
                    nc.vector.tensor_mul(t2_t[:, blk], t_t[:, blk], t_t[:, blk])
                qs_t = qpool.tile([128, SC], BF16, tag="qs")
                for j in range(SC // 512):
                    blk = slice(j * 512, (j + 1) * 512)
                    qps = pps.tile([128, 512], F32, tag="kfps")
                    nc.tensor.matmul(qps, lhsT=bmq_bf, rhs=t_t[:, blk],
                                     start=True, stop=False)
                    nc.tensor.matmul(qps, lhsT=fmat_bf, rhs=t2_t[:, blk],
                                     start=False, stop=True)
                    nc.scalar.activation(qs_t[:, blk], qps, AF.Exp)
                qs_tiles[isc] = qs_t

            def emit_q_num(isc, kvb):
                qs_t = qs_tiles.pop(isc)
                out_t = opool.tile([128, SC // 128, 128], F32, tag="outb")
                cc = 0
                while cc < SC // 128:
                    g = min(3, SC // 128 - cc)
                    nps = pps.tile([128, 3, 136], F32, tag="vps")
                    for i in range(g):
                        ch = slice((cc + i) * 128, (cc + i + 1) * 128)
                        nc.tensor.matmul(nps[:, i, :], lhsT=qs_t[:, ch], rhs=kvb,
                                         start=True, stop=True)
                    nps4 = nps[:, 0:g, :].rearrange("p c (h j) -> p c h j", h=8)
                    rden = npool.tile([128, 3, 8], F32, tag="rden")
                    nc.vector.reciprocal(rden[:, 0:g], nps4[:, :, :, 16])
                    nc.vector.tensor_mul(
                        out_t[:, cc:cc + g].rearrange("p c (h j) -> p c h j", h=8),
                        nps4[:, :, :, 0:16],
                        rden[:, 0:g, :, None].to_broadcast([128, g, 8, 16]))
                    cc += g
                    if cc == 9:
                        nc.sync.dma_start(out=outp_r[isc][:, 0:8], in_=out_t[:, 0:8])
                nc.sync.dma_start(out=outp_r[isc][:, 8:], in_=out_t[:, 8:])

            for isc in range(nkc):
                emit_k_sc(isc)
            kvb = emit_kvb()
            for isc in range(SQ // SC):
                emit_q_feat(isc)
                emit_q_num(isc, kvb)

        if loop_n is None:
            body()
        else:
            with tc.For_i(0, loop_n, 1):
                body()

    nc.compile()
    return nc


def _get_program():
    global _PROGRAM
    if _PROGRAM is None:
        _PROGRAM = _build_program(seqpar=SEQPAR)
    return _PROGRAM


def _host_consts(rfs, Wq, bq, Wk, bk):
    import ml_dtypes
    scale = HD ** -0.25  # == 0.5 exactly
    rfs = rfs.astype(np.float64)
    bmat = np.zeros((128, 128))
    fmat = np.zeros((128, 128))
    for h in range(N_HEADS):
        bmat[16 * h:16 * h + 16, 16 * h:16 * h + 16] = rfs[h]
        fmat[16 * h:16 * h + 16, 16 * h:16 * h + 16] = -0.5

    def fold(bvec):
        bp = scale * bvec.astype(np.float64)
        bm = bmat.copy()
        cc = np.zeros(128)
        for h in range(N_HEADS):
            sl = slice(16 * h, 16 * h + 16)
            bm[sl, sl] -= bp[sl][:, None]
            cc[sl] = bp @ bmat[:, sl] - 0.5 * np.sum(bp[sl] ** 2)
        return bm, cc

    bmq, cq = fold(bq)
    bmk, ck = fold(bk)
    csc = np.exp(cq + ck)

    mask = np.zeros((128, 136), dtype=np.float64)
    for h in range(N_HEADS):
        mask[16 * h:16 * h + 16, 17 * h:17 * h + 17] = 1.0
    mask *= csc[:, None]
    cpack = np.ascontiguousarray(mask, dtype=np.float32)

    fm8 = np.zeros((128, 8))
    for h in range(N_HEADS):
        fm8[16 * h:16 * h + 16, h] = -0.5
    cpackb = np.concatenate([bmq, bmk, fmat, fm8], axis=1).astype(ml_dtypes.bfloat16)
    cpackr = np.ascontiguousarray(np.concatenate(
        [(scale * Wq).T, (scale * Wk).T, np.eye(128)], axis=1), dtype=np.float32)
    return cpack, cpackb, cpackr


SEQPAR = False


def make_in_maps(inputs):
    x = np.ascontiguousarray(np.asarray(inputs["x"], dtype=np.float32))
    y = np.ascontiguousarray(np.asarray(inputs["y"], dtype=np.float32))
    cpack, cpackb, cpackr = _host_consts(np.asarray(inputs["rfs"], dtype=np.float32),
                         np.asarray(inputs["Wq"], dtype=np.float32),
                         np.asarray(inputs["bq"], dtype=np.float32),
                         np.asarray(inputs["Wk"], dtype=np.float32),
                         np.asarray(inputs["bk"], dtype=np.float32))
    B = x.shape[0]
    xr = x.reshape(B, C, S)
    yr = y.reshape(B, C, S)
    in_maps = []
    for core in range(8):
        b, half = core // 2, core % 2
        s0 = half * SQ
        ys_i = yr[b][:, s0:s0 + SQ] if SEQPAR else yr[b]
        in_maps.append({
            "xs": np.ascontiguousarray(xr[b][:, s0:s0 + SQ]),
            "ys": np.ascontiguousarray(ys_i),
            "cpack": cpack,
            "cpackb": cpackb,
            "cpackr": cpackr,
        })
    return in_maps


def run(inputs, trace=False, **kwargs):
    in_maps = make_in_maps(inputs)
    nc = _get_program()
    res = run_bass_kernel_spmd(nc, in_maps, list(range(8)), trace=trace, **kwargs)
    B = np.asarray(inputs["x"]).shape[0]
    out = np.empty((B, S, 128), dtype=np.float32)
    for core in range(8):
        b, half = core // 2, core % 2
        s0 = half * SQ
        out[b, s0:s0 + SQ, :] = res.results[core]["outp"]
    return out.reshape(np.asarray(inputs["x"]).shape), res


def kernel(**inputs):
    out, _ = run(inputs, trace=False)
    return out
